# revision 6
# baseline (speedup 1.0000x reference)
"""Trainium2 Bass kernel for the EvolutionaryFeatureExtractor problem.

Computes (pssm[512,20], conservation[512], mi_matrix[512,512]) from an MSA
[2048, 512] of int32 tokens (0..19 amino acids, 20 = gap) and a pseudocount
scale pc[1].

Strategy (8 NeuronCores, SPMD, no collectives):
  - MI pair work is sharded over i-positions: core c owns positions
    13c..13c+12 of the first 100 (core 7 carries 4 dummy positions that the
    host drops).  Each core computes J-rows = X_slice^T @ X for its slice,
    where X is the one-hot [2048, 2000] over the first 100 positions,
    via PE matmuls on a bf16 one-hot built on-chip with is_equal compares.
  - MI reduces to entropies:  mi = ((U - V - W)/tot + ln tot)/ln 2 with
      U = sum_ab J ln J, V = sum_b RS ln RS, W = sum_a CS ln CS,
      RS/CS the within-block marginals, tot the pair count.  RS rows are
    obtained for free by interleaving a non-gap-indicator column into the
    stationary operand (21 columns per position).
  - PSSM/conservation counts are sharded over the 512 columns (64 per core)
    and computed with a ones-row matmul over the one-hot.
Host side only slices inputs per core and concatenates the outputs.
"""

import numpy as np
from contextlib import ExitStack

import concourse.bass as bass
import concourse.bacc as bacc
import concourse.tile as tile
from concourse import mybir
from concourse.bass_utils import run_bass_kernel_spmd

# problem geometry (hardcoded per contest rules)
N_SEQS = 2048
SEQ_LEN = 512
NAA = 20
MPOS = 100          # MI over first 100 positions
NCORES = 8
POS_PER_CORE = 13   # 8*13 = 104 >= 100 (4 dummies on core 7)
CNT_PER_CORE = 64   # 512/8
P = 128
KCH = N_SEQS // P   # 16 K-chunks
NW = NAA + 1        # 20 one-hot cols + 1 non-gap col per position
LN2 = float(np.log(2.0))
LN20 = float(np.log(20.0))
EPS = 1e-10

f32 = mybir.dt.float32
bf16 = mybir.dt.bfloat16
i32 = mybir.dt.int32
Alu = mybir.AluOpType
Act = mybir.ActivationFunctionType

# M-tiles: position-aligned groups of the 13 owned positions
MT = [(0, 6), (6, 6), (12, 1)]


def _emit_kernel(nc, tc, ctx, tensors):
    (msa100, msa_mi, msa_cnt, s_all, dmask, pc,
     pssm_o, cons_o, mi_o) = tensors

    consts = ctx.enter_context(tc.tile_pool(name="consts", bufs=1))
    xp = ctx.enter_context(tc.tile_pool(name="xp", bufs=1))
    post = ctx.enter_context(tc.tile_pool(name="post", bufs=1))
    small = ctx.enter_context(tc.tile_pool(name="small", bufs=2))
    jpsum = ctx.enter_context(tc.tile_pool(name="jpsum", bufs=1, space="PSUM"))
    ppsum = ctx.enter_context(tc.tile_pool(name="ppsum", bufs=1, space="PSUM"))
    dpool = ctx.enter_context(tc.tile_pool(name="dscratch", bufs=1, space="DRAM"))

    KG = 4               # k-chunks per pipeline group
    NG = KCH // KG       # 4 groups

    # ---------------- small constant loads ----------------
    s_sb = consts.tile([P, 12], f32)
    nc.sync.dma_start(out=s_sb[:], in_=s_all[:, :])
    dm = []
    for t, (p0, npos) in enumerate(MT):
        d = consts.tile([npos, MPOS], f32, tag=f"dm{t}")
        nc.sync.dma_start(out=d[:], in_=dmask[p0:p0 + npos, :])
        dm.append(d)
    ones_sb = consts.tile([P, 1], bf16)
    nc.vector.memset(ones_sb[:], 1.0)
    eps_sb = consts.tile([P, 1], f32)
    nc.vector.memset(eps_sb[:], EPS)

    # ---------------- stationary one-hot (small, built first) ----------------
    msa_mi_i = consts.tile([P, KCH, POS_PER_CORE], i32)
    nc.sync.dma_start(out=msa_mi_i[:], in_=msa_mi[:, :].rearrange("(k p) i -> p k i", p=P))
    msa_mi_bf = consts.tile([P, KCH, POS_PER_CORE], bf16)
    nc.gpsimd.tensor_copy(out=msa_mi_bf[:], in_=msa_mi_i[:])
    xstat = xp.tile([P, KCH, POS_PER_CORE, NW], bf16)
    for a in range(NAA):
        nc.vector.tensor_scalar(out=xstat[:, :, :, a], in0=msa_mi_bf[:],
                                scalar1=float(a), scalar2=None, op0=Alu.is_equal)
    nc.vector.tensor_scalar(out=xstat[:, :, :, NAA], in0=msa_mi_bf[:],
                            scalar1=float(NAA), scalar2=None, op0=Alu.is_lt)

    # ---------------- moving one-hot, pipelined per k-group ----------------
    msa100_i = consts.tile([P, KCH, MPOS], i32)
    msa100_bf = consts.tile([P, KCH, MPOS], bf16)
    msa100_r = msa100[:, :].rearrange("(k p) i -> p k i", p=P)
    xmov = xp.tile([P, NAA, KCH, MPOS], bf16)
    for g in range(NG):
        k0 = g * KG
        eng = nc.sync if g % 2 == 0 else nc.scalar
        eng.dma_start(out=msa100_i[:, k0:k0 + KG, :], in_=msa100_r[:, k0:k0 + KG, :])
        nc.gpsimd.tensor_copy(out=msa100_bf[:, k0:k0 + KG, :],
                              in_=msa100_i[:, k0:k0 + KG, :])
        for a in range(NAA):
            nc.vector.tensor_scalar(out=xmov[:, a, k0:k0 + KG, :],
                                    in0=msa100_bf[:, k0:k0 + KG, :],
                                    scalar1=float(a), scalar2=None, op0=Alu.is_equal)

    # ---------------- counts one-hot (overlaps J-mt0 matmuls) ----------------
    msa_cnt_i = consts.tile([P, KCH, CNT_PER_CORE], i32)
    nc.scalar.dma_start(out=msa_cnt_i[:], in_=msa_cnt[:, :].rearrange("(k p) i -> p k i", p=P))
    msa_cnt_bf = consts.tile([P, KCH, CNT_PER_CORE], bf16)
    nc.gpsimd.tensor_copy(out=msa_cnt_bf[:], in_=msa_cnt_i[:])
    xcnt = xp.tile([P, NAA, KCH, CNT_PER_CORE], bf16)
    for a in range(NAA):
        nc.vector.tensor_scalar(out=xcnt[:, a, :, :], in0=msa_cnt_bf[:],
                                scalar1=float(a), scalar2=None, op0=Alu.is_equal)

    # ---------------- J matmuls + MI post, per M-tile ----------------
    def emit_mt(t):
        p0, npos = MT[t]
        mr = npos * NW
        jps = jpsum.tile([126, 4, 512], f32, tag="jps")
        for k in range(KCH):
            lhsT = xstat[:, k, p0:p0 + npos, :]
            for n in range(4):
                nc.tensor.matmul(jps[0:mr, n, 0:500], lhsT=lhsT,
                                 rhs=xmov[:, 5 * n:5 * n + 5, k, :],
                                 start=(k == 0), stop=(k == KCH - 1))
        # copy PSUM -> SBUF (flat a-major 2000 cols) split over ACT/DVE;
        # frees the psum slot quickly for the next user
        jsb = post.tile([126, 2000], f32, tag="jsb")
        jview = jsb[0:mr].rearrange("p (n c) -> p n c", n=4)
        nc.scalar.copy(out=jview[:, 0:2, :], in_=jps[0:mr, 0:2, 0:500])
        nc.vector.tensor_copy(out=jview[:, 2:4, :], in_=jps[0:mr, 2:4, 0:500])
        # L = ln(J + eps);  E = J * L (gpsimd);  segmented sums over b
        lnj = post.tile([126, 2000], f32, tag="lnj")
        nc.scalar.activation(out=lnj[0:mr], in_=jsb[0:mr], func=Act.Ln,
                             bias=eps_sb[0:mr, 0:1], scale=1.0)
        ee = post.tile([126, 2000], f32, tag="ee")
        nc.gpsimd.tensor_tensor(ee[0:mr], jsb[0:mr], lnj[0:mr], Alu.mult)
        eucg = post.tile([126, 3, MPOS], f32, tag="eucg")
        nc.vector.tensor_reduce(out=eucg[0:mr, 0, :],
                                in_=ee[0:mr].rearrange("p (b j) -> p j b", b=NAA),
                                axis=mybir.AxisListType.X, op=Alu.add)
        nc.vector.tensor_reduce(out=eucg[0:mr, 1, :],
                                in_=jsb[0:mr].rearrange("p (b j) -> p j b", b=NAA),
                                axis=mybir.AxisListType.X, op=Alu.add)
        lncs = post.tile([126, MPOS], f32, tag="lncs")
        nc.scalar.activation(out=lncs[0:mr], in_=eucg[0:mr, 1, :], func=Act.Ln,
                             bias=eps_sb[0:mr, 0:1], scale=1.0)
        nc.vector.tensor_tensor(eucg[0:mr, 2, :], eucg[0:mr, 1, :], lncs[0:mr], Alu.mult)

        # group sums via small matmuls:  psU rows=[U|-|W], psV rows=[V|tot|-]
        psU = ppsum.tile([6, 3, MPOS], f32, tag="psU")
        psV = ppsum.tile([6, 3, MPOS], f32, tag="psV")
        if t < 2:
            sa, sn = s_sb[0:mr, 0:6], s_sb[0:mr, 6:12]
        else:
            sa, sn = s_sb[0:mr, 0:1], s_sb[0:mr, 6:7]
        nc.tensor.matmul(psU[0:npos, :, :], lhsT=sa, rhs=eucg[0:mr, :, :],
                         start=True, stop=True)
        nc.tensor.matmul(psV[0:npos, :, :], lhsT=sn, rhs=eucg[0:mr, :, :],
                         start=True, stop=True)

        # mi = ((U - V - W) / max(tot,1) + ln tot) * dmask/ln2
        vt = small.tile([6, 2, MPOS], f32, tag="vt")
        nc.scalar.copy(out=vt[0:npos, :, :], in_=psV[0:npos, 0:2, :])
        tts = small.tile([6, MPOS], f32, tag="tts")
        nc.vector.tensor_scalar(out=tts[0:npos], in0=vt[0:npos, 1, :], scalar1=1.0,
                                scalar2=None, op0=Alu.max)
        inv = small.tile([6, MPOS], f32, tag="inv")
        nc.vector.reciprocal(out=inv[0:npos], in_=tts[0:npos])
        lnt = small.tile([6, MPOS], f32, tag="lnt")
        nc.scalar.activation(out=lnt[0:npos], in_=tts[0:npos], func=Act.Ln,
                             bias=0.0, scale=1.0)
        acc = small.tile([6, MPOS], f32, tag="acc")
        nc.vector.tensor_tensor(acc[0:npos], psU[0:npos, 0, :], vt[0:npos, 0, :], Alu.subtract)
        nc.vector.tensor_tensor(acc[0:npos], acc[0:npos], psU[0:npos, 2, :], Alu.subtract)
        nc.vector.tensor_tensor(acc[0:npos], acc[0:npos], inv[0:npos], Alu.mult)
        nc.vector.tensor_tensor(acc[0:npos], acc[0:npos], lnt[0:npos], Alu.add)
        nc.vector.tensor_tensor(acc[0:npos], acc[0:npos], dm[t][0:npos, :], Alu.mult)
        nc.sync.dma_start(out=mi_o[p0:p0 + npos, :], in_=acc[0:npos])

    emit_mt(0)

    # ---------------- counts matmul (ones row), between mt0 and mt1 ----------
    cnt_ps = jpsum.tile([1, 3, 512], f32, tag="jps")
    CNT_NT = [(0, 8), (8, 8), (16, 4)]
    for k in range(KCH):
        for ni, (a0, aw) in enumerate(CNT_NT):
            nc.tensor.matmul(cnt_ps[0:1, ni, 0:aw * CNT_PER_CORE],
                             lhsT=ones_sb[:, 0:1],
                             rhs=xcnt[:, a0:a0 + aw, k, :],
                             start=(k == 0), stop=(k == KCH - 1))
    cnts = post.tile([1, 1280], f32)
    nc.scalar.copy(out=cnts[:],
                   in_=cnt_ps[0:1, :, :].rearrange("p a b -> p (a b)")[:, 0:1280])
    cnt_dram = dpool.tile([CNT_PER_CORE, NAA], f32)
    nc.gpsimd.dma_start(out=cnt_dram[:, :].rearrange("l a -> a l"),
                        in_=cnts[0:1, :].rearrange("p (a l) -> p a l", a=NAA))
    cnt64 = post.tile([CNT_PER_CORE, NAA], f32)
    nc.gpsimd.dma_start(out=cnt64[:], in_=cnt_dram[:, :])

    emit_mt(1)

    # ---------------- pssm ----------------
    pcb = small.tile([CNT_PER_CORE, 1], f32)
    nc.gpsimd.dma_start(out=pcb[:], in_=pc[:, :].broadcast_to([CNT_PER_CORE, 1]))
    den = small.tile([CNT_PER_CORE, 1], f32)
    # den = pc*0.2 + 2048  ( = 2048 + 20*pseudocount, pseudocount = 0.01*pc )
    nc.vector.tensor_scalar(out=den[:], in0=pcb[:], scalar1=0.2, scalar2=2048.0,
                            op0=Alu.mult, op1=Alu.add)
    invd = small.tile([CNT_PER_CORE, 1], f32)
    nc.vector.reciprocal(out=invd[:], in_=den[:])
    sc = small.tile([CNT_PER_CORE, 1], f32)
    nc.vector.tensor_scalar(out=sc[:], in0=invd[:], scalar1=20.0, scalar2=None, op0=Alu.mult)
    pcntb = small.tile([CNT_PER_CORE, 1], f32)
    nc.vector.tensor_scalar(out=pcntb[:], in0=pcb[:], scalar1=0.01, scalar2=None, op0=Alu.mult)
    cntp = small.tile([CNT_PER_CORE, NAA], f32)
    nc.vector.tensor_scalar(out=cntp[:], in0=cnt64[:], scalar1=pcntb[:, 0:1],
                            scalar2=None, op0=Alu.add)
    pssm_sb = small.tile([CNT_PER_CORE, NAA], f32)
    nc.scalar.activation(out=pssm_sb[:], in_=cntp[:], func=Act.Ln,
                         bias=eps_sb[0:CNT_PER_CORE, 0:1], scale=sc[:, 0:1])
    nc.sync.dma_start(out=pssm_o[:, :], in_=pssm_sb[:])

    # ---------------- conservation ----------------
    total = small.tile([CNT_PER_CORE, 1], f32)
    nc.vector.tensor_reduce(out=total[:], in_=cnt64[:], axis=mybir.AxisListType.X, op=Alu.add)
    tots = small.tile([CNT_PER_CORE, 1], f32)
    nc.vector.tensor_scalar(out=tots[:], in0=total[:], scalar1=1.0, scalar2=None, op0=Alu.max)
    invt = small.tile([CNT_PER_CORE, 1], f32)
    nc.vector.reciprocal(out=invt[:], in_=tots[:])
    ffreq = small.tile([CNT_PER_CORE, NAA], f32)
    nc.vector.tensor_scalar(out=ffreq[:], in0=cnt64[:], scalar1=invt[:, 0:1],
                            scalar2=None, op0=Alu.mult)
    lf = small.tile([CNT_PER_CORE, NAA], f32)
    nc.scalar.activation(out=lf[:], in_=ffreq[:], func=Act.Ln,
                         bias=eps_sb[0:CNT_PER_CORE, 0:1], scale=1.0)
    fl = small.tile([CNT_PER_CORE, NAA], f32)
    nc.vector.tensor_tensor(fl[:], ffreq[:], lf[:], Alu.mult)
    se = small.tile([CNT_PER_CORE, 1], f32)
    nc.vector.tensor_reduce(out=se[:], in_=fl[:], axis=mybir.AxisListType.X, op=Alu.add)
    consv = small.tile([CNT_PER_CORE, 1], f32)
    # cons = 1 + (sum f ln f)/ln(20)
    nc.vector.tensor_scalar(out=consv[:], in0=se[:], scalar1=1.0 / LN20, scalar2=1.0,
                            op0=Alu.mult, op1=Alu.add)
    mask = small.tile([CNT_PER_CORE, 1], f32)
    nc.vector.tensor_scalar(out=mask[:], in0=total[:], scalar1=0.0, scalar2=None, op0=Alu.is_gt)
    nc.vector.tensor_tensor(consv[:], consv[:], mask[:], Alu.mult)
    nc.sync.dma_start(out=cons_o[:, :], in_=consv[:])

    emit_mt(2)


_NC_CACHE = None


def _build_nc():
    global _NC_CACHE
    if _NC_CACHE is not None:
        return _NC_CACHE
    nc = bacc.Bacc("TRN2", target_bir_lowering=False)
    msa100 = nc.dram_tensor("msa100", [N_SEQS, MPOS], i32, kind="ExternalInput")
    msa_mi = nc.dram_tensor("msa_mi", [N_SEQS, POS_PER_CORE], i32, kind="ExternalInput")
    msa_cnt = nc.dram_tensor("msa_cnt", [N_SEQS, CNT_PER_CORE], i32, kind="ExternalInput")
    s_all = nc.dram_tensor("s_all", [P, 12], f32, kind="ExternalInput")
    dmask = nc.dram_tensor("dmask", [POS_PER_CORE, MPOS], f32, kind="ExternalInput")
    pc = nc.dram_tensor("pc", [1, 1], f32, kind="ExternalInput")
    pssm_o = nc.dram_tensor("pssm_part", [CNT_PER_CORE, NAA], f32, kind="ExternalOutput")
    cons_o = nc.dram_tensor("cons_part", [CNT_PER_CORE, 1], f32, kind="ExternalOutput")
    mi_o = nc.dram_tensor("mi_part", [POS_PER_CORE, MPOS], f32, kind="ExternalOutput")
    with tile.TileContext(nc) as tc:
        with ExitStack() as ctx:
            _emit_kernel(nc, tc, ctx,
                         (msa100, msa_mi, msa_cnt, s_all, dmask, pc,
                          pssm_o, cons_o, mi_o))
    nc.compile()
    _NC_CACHE = nc
    return nc


def _host_inputs(msa, pc):
    msa = np.ascontiguousarray(np.asarray(msa), dtype=np.int32)
    pc_np = np.asarray(pc, dtype=np.float32).reshape(1, 1)
    s_arr = np.zeros((P, 12), np.float32)
    for m in range(6):
        s_arr[NW * m: NW * m + NAA, m] = 1.0
        s_arr[NW * m + NAA, 6 + m] = 1.0
    msa100 = np.ascontiguousarray(msa[:, :MPOS])
    in_maps = []
    for c in range(NCORES):
        cols = [(POS_PER_CORE * c + t) if (POS_PER_CORE * c + t) < MPOS else 0
                for t in range(POS_PER_CORE)]
        dmask = np.full((POS_PER_CORE, MPOS), 1.0 / LN2, np.float32)
        for t in range(POS_PER_CORE):
            g = POS_PER_CORE * c + t
            if g < MPOS:
                dmask[t, g] = 0.0
        in_maps.append({
            "msa100": msa100,
            "msa_mi": np.ascontiguousarray(msa[:, cols]),
            "msa_cnt": np.ascontiguousarray(msa[:, CNT_PER_CORE * c: CNT_PER_CORE * (c + 1)]),
            "s_all": s_arr,
            "dmask": dmask,
            "pc": pc_np,
        })
    return in_maps


def _run(msa, pc, **spmd_kwargs):
    nc = _build_nc()
    in_maps = _host_inputs(msa, pc)
    res = run_bass_kernel_spmd(nc, in_maps, core_ids=list(range(NCORES)), **spmd_kwargs)
    pssm = np.concatenate([res.results[c]["pssm_part"] for c in range(NCORES)], axis=0)
    cons = np.concatenate([res.results[c]["cons_part"][:, 0] for c in range(NCORES)], axis=0)
    rows = np.concatenate([res.results[c]["mi_part"] for c in range(NCORES)], axis=0)
    mi = np.zeros((SEQ_LEN, SEQ_LEN), np.float32)
    mi[:MPOS, :MPOS] = rows[:MPOS]
    return (pssm.astype(np.float32), cons.astype(np.float32), mi), res


def kernel(msa, pc):
    out, _ = _run(msa, pc)
    return out


# revision 7
# speedup vs baseline: 1.1933x; 1.1933x over previous
"""Trainium2 Bass kernel for the EvolutionaryFeatureExtractor problem.

Computes (pssm[512,20], conservation[512], mi_matrix[512,512]) from an MSA
[2048, 512] of int32 tokens (0..19 amino acids, 20 = gap) and a pseudocount
scale pc[1].

Strategy (8 NeuronCores, SPMD, no collectives):
  - MI pair work is sharded over i-positions: core c owns positions
    13c..13c+12 of the first 100 (core 7 carries 4 dummy positions that the
    host drops).  Each core computes J-rows = X_slice^T @ X for its slice,
    where X is the one-hot [2048, 2000] over the first 100 positions,
    via PE matmuls on a bf16 one-hot built on-chip with is_equal compares.
  - MI reduces to entropies:  mi = ((U - V - W)/tot + ln tot)/ln 2 with
      U = sum_ab J ln J, V = sum_b RS ln RS, W = sum_a CS ln CS,
      RS/CS the within-block marginals, tot the pair count.  RS rows are
    obtained for free by interleaving a non-gap-indicator column into the
    stationary operand (21 columns per position).
  - PSSM/conservation counts are sharded over the 512 columns (64 per core)
    and computed with a ones-row matmul over the one-hot.
Host side only slices inputs per core and concatenates the outputs.
"""

import numpy as np
from contextlib import ExitStack

import concourse.bass as bass
import concourse.bacc as bacc
import concourse.tile as tile
from concourse import mybir
from concourse.bass_utils import run_bass_kernel_spmd

# problem geometry (hardcoded per contest rules)
N_SEQS = 2048
SEQ_LEN = 512
NAA = 20
MPOS = 100          # MI over first 100 positions
NCORES = 8
POS_PER_CORE = 13   # 8*13 = 104 >= 100 (4 dummies on core 7)
CNT_PER_CORE = 64   # 512/8
P = 128
KCH = N_SEQS // P   # 16 K-chunks
NW = NAA + 1        # 20 one-hot cols + 1 non-gap col per position
LN2 = float(np.log(2.0))
LN20 = float(np.log(20.0))
EPS = 1e-10

f32 = mybir.dt.float32
bf16 = mybir.dt.bfloat16
i32 = mybir.dt.int32
Alu = mybir.AluOpType
Act = mybir.ActivationFunctionType

# M-tiles: position-aligned groups of the 13 owned positions
MT = [(0, 6), (6, 6), (12, 1)]


def _emit_kernel(nc, tc, ctx, tensors):
    (msa100, msa_mi, msa_cnt, s_all, dmask, pc,
     pssm_o, cons_o, mi_o) = tensors

    consts = ctx.enter_context(tc.tile_pool(name="consts", bufs=1))
    xp = ctx.enter_context(tc.tile_pool(name="xp", bufs=1))
    post = ctx.enter_context(tc.tile_pool(name="post", bufs=1))
    small = ctx.enter_context(tc.tile_pool(name="small", bufs=2))
    jpsum = ctx.enter_context(tc.tile_pool(name="jpsum", bufs=1, space="PSUM"))
    cspsum = ctx.enter_context(tc.tile_pool(name="cspsum", bufs=1, space="PSUM"))
    ppsum = ctx.enter_context(tc.tile_pool(name="ppsum", bufs=1, space="PSUM"))
    dpool = ctx.enter_context(tc.tile_pool(name="dscratch", bufs=1, space="DRAM"))

    KG = 4               # k-chunks per pipeline group
    NG = KCH // KG       # 4 groups

    # ---------------- small constant loads ----------------
    s_sb = consts.tile([P, 12], f32)
    nc.sync.dma_start(out=s_sb[:], in_=s_all[:, :])
    dm = []
    for t, (p0, npos) in enumerate(MT):
        d = consts.tile([npos, MPOS], f32, tag=f"dm{t}")
        nc.sync.dma_start(out=d[:], in_=dmask[p0:p0 + npos, :])
        dm.append(d)
    ones_sb = consts.tile([P, 1], bf16)
    nc.vector.memset(ones_sb[:], 1.0)
    eps_sb = consts.tile([P, 1], f32)
    nc.vector.memset(eps_sb[:], EPS)

    # ---------------- stationary one-hot (small, built first) ----------------
    msa_mi_i = consts.tile([P, KCH, POS_PER_CORE], i32)
    nc.sync.dma_start(out=msa_mi_i[:], in_=msa_mi[:, :].rearrange("(k p) i -> p k i", p=P))
    msa_mi_bf = consts.tile([P, KCH, POS_PER_CORE], bf16)
    nc.vector.tensor_copy(out=msa_mi_bf[:], in_=msa_mi_i[:])
    xstat = xp.tile([P, KCH, POS_PER_CORE, NW], bf16)
    for a in range(NAA):
        nc.vector.tensor_scalar(out=xstat[:, :, :, a], in0=msa_mi_bf[:],
                                scalar1=float(a), scalar2=None, op0=Alu.is_equal)
    nc.vector.tensor_scalar(out=xstat[:, :, :, NAA], in0=msa_mi_bf[:],
                            scalar1=float(NAA), scalar2=None, op0=Alu.is_lt)

    # ---------------- moving one-hot, pipelined per k-group ----------------
    msa100_i = consts.tile([P, KCH, MPOS], i32)
    msa100_bf = consts.tile([P, KCH, MPOS], bf16)
    msa100_r = msa100[:, :].rearrange("(k p) i -> p k i", p=P)
    xmov = xp.tile([P, NAA, KCH, MPOS], bf16)
    xng = xp.tile([P, KCH, MPOS], bf16)

    def build_kg(g):
        k0 = g * KG
        eng = nc.sync if g % 2 == 0 else nc.scalar
        eng.dma_start(out=msa100_i[:, k0:k0 + KG, :], in_=msa100_r[:, k0:k0 + KG, :])
        nc.vector.tensor_copy(out=msa100_bf[:, k0:k0 + KG, :],
                              in_=msa100_i[:, k0:k0 + KG, :])
        for a in range(NAA):
            nc.vector.tensor_scalar(out=xmov[:, a, k0:k0 + KG, :],
                                    in0=msa100_bf[:, k0:k0 + KG, :],
                                    scalar1=float(a), scalar2=None, op0=Alu.is_equal)
        nc.vector.tensor_scalar(out=xng[:, k0:k0 + KG, :],
                                in0=msa100_bf[:, k0:k0 + KG, :],
                                scalar1=float(NAA), scalar2=None, op0=Alu.is_lt)

    # ---------------- J matmuls + MI post, per M-tile ----------------
    def emit_mt_matmuls(t, interleave_builds=False):
        p0, npos = MT[t]
        mr = npos * NW
        jps = jpsum.tile([126, 4, 512], f32, tag="jps")
        csps = cspsum.tile([126, 128], f32, tag="csps")
        for k in range(KCH):
            if interleave_builds and k % KG == 0:
                build_kg(k // KG)
            lhsT = xstat[:, k, p0:p0 + npos, :]
            for n in range(4):
                nc.tensor.matmul(jps[0:mr, n, 0:500], lhsT=lhsT,
                                 rhs=xmov[:, 5 * n:5 * n + 5, k, :],
                                 start=(k == 0), stop=(k == KCH - 1))
            nc.tensor.matmul(csps[0:mr, 0:MPOS], lhsT=lhsT, rhs=xng[:, k, :],
                             start=(k == 0), stop=(k == KCH - 1))
        return jps, csps

    def emit_mt_post(t, jps, csps):
        p0, npos = MT[t]
        mr = npos * NW
        # copy PSUM -> SBUF (flat a-major 2000 cols) split over ACT/DVE;
        # frees the psum slot quickly for the next user
        jsb = post.tile([126, 2000], f32, tag="jsb")
        jview = jsb[0:mr].rearrange("p (n c) -> p n c", n=4)
        nc.scalar.copy(out=jview[:, 0:2, :], in_=jps[0:mr, 0:2, 0:500])
        nc.vector.tensor_copy(out=jview[:, 2:4, :], in_=jps[0:mr, 2:4, 0:500])
        eucg = post.tile([126, 3, MPOS], f32, tag="eucg")
        nc.scalar.copy(out=eucg[0:mr, 1, :], in_=csps[0:mr, 0:MPOS])
        # L = ln(J + eps);  E = J * L;  segmented sum over b
        lnj = post.tile([126, 2000], f32, tag="lnj")
        nc.scalar.activation(out=lnj[0:mr], in_=jsb[0:mr], func=Act.Ln,
                             bias=eps_sb[0:mr, 0:1], scale=1.0)
        ee = post.tile([126, 2000], f32, tag="ee")
        nc.vector.tensor_tensor(ee[0:mr], jsb[0:mr], lnj[0:mr], Alu.mult)
        nc.vector.tensor_reduce(out=eucg[0:mr, 0, :],
                                in_=ee[0:mr].rearrange("p (b j) -> p j b", b=NAA),
                                axis=mybir.AxisListType.X, op=Alu.add)
        lncs = post.tile([126, MPOS], f32, tag="lncs")
        nc.scalar.activation(out=lncs[0:mr], in_=eucg[0:mr, 1, :], func=Act.Ln,
                             bias=eps_sb[0:mr, 0:1], scale=1.0)
        nc.vector.tensor_tensor(eucg[0:mr, 2, :], eucg[0:mr, 1, :], lncs[0:mr], Alu.mult)

        # group sums via small matmuls:  psU rows=[U|-|W], psV rows=[V|tot|-]
        psU = ppsum.tile([6, 3, MPOS], f32, tag="psU")
        psV = ppsum.tile([6, 3, MPOS], f32, tag="psV")
        if t < 2:
            sa, sn = s_sb[0:mr, 0:6], s_sb[0:mr, 6:12]
        else:
            sa, sn = s_sb[0:mr, 0:1], s_sb[0:mr, 6:7]
        nc.tensor.matmul(psU[0:npos, :, :], lhsT=sa, rhs=eucg[0:mr, :, :],
                         start=True, stop=True)
        nc.tensor.matmul(psV[0:npos, :, :], lhsT=sn, rhs=eucg[0:mr, :, :],
                         start=True, stop=True)

        # mi = ((U - V - W) / max(tot,1) + ln tot) * dmask/ln2
        vt = small.tile([6, 2, MPOS], f32, tag="vt")
        nc.scalar.copy(out=vt[0:npos, :, :], in_=psV[0:npos, 0:2, :])
        tts = small.tile([6, MPOS], f32, tag="tts")
        nc.vector.tensor_scalar(out=tts[0:npos], in0=vt[0:npos, 1, :], scalar1=1.0,
                                scalar2=None, op0=Alu.max)
        inv = small.tile([6, MPOS], f32, tag="inv")
        nc.vector.reciprocal(out=inv[0:npos], in_=tts[0:npos])
        lnt = small.tile([6, MPOS], f32, tag="lnt")
        nc.scalar.activation(out=lnt[0:npos], in_=tts[0:npos], func=Act.Ln,
                             bias=0.0, scale=1.0)
        acc = small.tile([6, MPOS], f32, tag="acc")
        nc.vector.tensor_tensor(acc[0:npos], psU[0:npos, 0, :], vt[0:npos, 0, :], Alu.subtract)
        nc.vector.tensor_tensor(acc[0:npos], acc[0:npos], psU[0:npos, 2, :], Alu.subtract)
        nc.vector.tensor_tensor(acc[0:npos], acc[0:npos], inv[0:npos], Alu.mult)
        nc.vector.tensor_tensor(acc[0:npos], acc[0:npos], lnt[0:npos], Alu.add)
        nc.vector.tensor_tensor(acc[0:npos], acc[0:npos], dm[t][0:npos, :], Alu.mult)
        nc.sync.dma_start(out=mi_o[p0:p0 + npos, :], in_=acc[0:npos])

    # mt0 with builds interleaved into its k-loop
    jps0, csps0 = emit_mt_matmuls(0, interleave_builds=True)
    emit_mt_post(0, jps0, csps0)

    # mt1 (xmov/xng fully built by now)
    jps1, csps1 = emit_mt_matmuls(1)

    # counts one-hot build (overlaps mt1 PE work)
    msa_cnt_i = consts.tile([P, KCH, CNT_PER_CORE], i32)
    nc.scalar.dma_start(out=msa_cnt_i[:], in_=msa_cnt[:, :].rearrange("(k p) i -> p k i", p=P))
    msa_cnt_bf = consts.tile([P, KCH, CNT_PER_CORE], bf16)
    nc.vector.tensor_copy(out=msa_cnt_bf[:], in_=msa_cnt_i[:])
    xcnt = xp.tile([P, NAA, KCH, CNT_PER_CORE], bf16)
    for a in range(NAA):
        nc.vector.tensor_scalar(out=xcnt[:, a, :, :], in0=msa_cnt_bf[:],
                                scalar1=float(a), scalar2=None, op0=Alu.is_equal)

    emit_mt_post(1, jps1, csps1)

    # ---------------- counts matmul (ones row), between mt1 and mt2 ----------
    cnt_ps = jpsum.tile([1, 3, 512], f32, tag="jps")
    CNT_NT = [(0, 8), (8, 8), (16, 4)]
    for k in range(KCH):
        for ni, (a0, aw) in enumerate(CNT_NT):
            nc.tensor.matmul(cnt_ps[0:1, ni, 0:aw * CNT_PER_CORE],
                             lhsT=ones_sb[:, 0:1],
                             rhs=xcnt[:, a0:a0 + aw, k, :],
                             start=(k == 0), stop=(k == KCH - 1))
    cnts = post.tile([1, 1280], f32)
    nc.scalar.copy(out=cnts[:],
                   in_=cnt_ps[0:1, :, :].rearrange("p a b -> p (a b)")[:, 0:1280])
    cnt_dram = dpool.tile([CNT_PER_CORE, NAA], f32)
    nc.gpsimd.dma_start(out=cnt_dram[:, :].rearrange("l a -> a l"),
                        in_=cnts[0:1, :].rearrange("p (a l) -> p a l", a=NAA))
    cnt64 = post.tile([CNT_PER_CORE, NAA], f32)
    nc.gpsimd.dma_start(out=cnt64[:], in_=cnt_dram[:, :])

    # mt2
    jps2, csps2 = emit_mt_matmuls(2)

    # ---------------- pssm ----------------
    pcb = small.tile([CNT_PER_CORE, 1], f32)
    nc.gpsimd.dma_start(out=pcb[:], in_=pc[:, :].broadcast_to([CNT_PER_CORE, 1]))
    den = small.tile([CNT_PER_CORE, 1], f32)
    # den = pc*0.2 + 2048  ( = 2048 + 20*pseudocount, pseudocount = 0.01*pc )
    nc.vector.tensor_scalar(out=den[:], in0=pcb[:], scalar1=0.2, scalar2=2048.0,
                            op0=Alu.mult, op1=Alu.add)
    invd = small.tile([CNT_PER_CORE, 1], f32)
    nc.vector.reciprocal(out=invd[:], in_=den[:])
    sc = small.tile([CNT_PER_CORE, 1], f32)
    nc.vector.tensor_scalar(out=sc[:], in0=invd[:], scalar1=20.0, scalar2=None, op0=Alu.mult)
    pcntb = small.tile([CNT_PER_CORE, 1], f32)
    nc.vector.tensor_scalar(out=pcntb[:], in0=pcb[:], scalar1=0.01, scalar2=None, op0=Alu.mult)
    cntp = small.tile([CNT_PER_CORE, NAA], f32)
    nc.vector.tensor_scalar(out=cntp[:], in0=cnt64[:], scalar1=pcntb[:, 0:1],
                            scalar2=None, op0=Alu.add)
    pssm_sb = small.tile([CNT_PER_CORE, NAA], f32)
    nc.scalar.activation(out=pssm_sb[:], in_=cntp[:], func=Act.Ln,
                         bias=eps_sb[0:CNT_PER_CORE, 0:1], scale=sc[:, 0:1])
    nc.sync.dma_start(out=pssm_o[:, :], in_=pssm_sb[:])

    # ---------------- conservation ----------------
    total = small.tile([CNT_PER_CORE, 1], f32)
    nc.vector.tensor_reduce(out=total[:], in_=cnt64[:], axis=mybir.AxisListType.X, op=Alu.add)
    tots = small.tile([CNT_PER_CORE, 1], f32)
    nc.vector.tensor_scalar(out=tots[:], in0=total[:], scalar1=1.0, scalar2=None, op0=Alu.max)
    invt = small.tile([CNT_PER_CORE, 1], f32)
    nc.vector.reciprocal(out=invt[:], in_=tots[:])
    ffreq = small.tile([CNT_PER_CORE, NAA], f32)
    nc.vector.tensor_scalar(out=ffreq[:], in0=cnt64[:], scalar1=invt[:, 0:1],
                            scalar2=None, op0=Alu.mult)
    lf = small.tile([CNT_PER_CORE, NAA], f32)
    nc.scalar.activation(out=lf[:], in_=ffreq[:], func=Act.Ln,
                         bias=eps_sb[0:CNT_PER_CORE, 0:1], scale=1.0)
    fl = small.tile([CNT_PER_CORE, NAA], f32)
    nc.vector.tensor_tensor(fl[:], ffreq[:], lf[:], Alu.mult)
    se = small.tile([CNT_PER_CORE, 1], f32)
    nc.vector.tensor_reduce(out=se[:], in_=fl[:], axis=mybir.AxisListType.X, op=Alu.add)
    consv = small.tile([CNT_PER_CORE, 1], f32)
    # cons = 1 + (sum f ln f)/ln(20)
    nc.vector.tensor_scalar(out=consv[:], in0=se[:], scalar1=1.0 / LN20, scalar2=1.0,
                            op0=Alu.mult, op1=Alu.add)
    mask = small.tile([CNT_PER_CORE, 1], f32)
    nc.vector.tensor_scalar(out=mask[:], in0=total[:], scalar1=0.0, scalar2=None, op0=Alu.is_gt)
    nc.vector.tensor_tensor(consv[:], consv[:], mask[:], Alu.mult)
    nc.sync.dma_start(out=cons_o[:, :], in_=consv[:])

    emit_mt_post(2, jps2, csps2)


_NC_CACHE = None


def _build_nc():
    global _NC_CACHE
    if _NC_CACHE is not None:
        return _NC_CACHE
    nc = bacc.Bacc("TRN2", target_bir_lowering=False)
    msa100 = nc.dram_tensor("msa100", [N_SEQS, MPOS], i32, kind="ExternalInput")
    msa_mi = nc.dram_tensor("msa_mi", [N_SEQS, POS_PER_CORE], i32, kind="ExternalInput")
    msa_cnt = nc.dram_tensor("msa_cnt", [N_SEQS, CNT_PER_CORE], i32, kind="ExternalInput")
    s_all = nc.dram_tensor("s_all", [P, 12], f32, kind="ExternalInput")
    dmask = nc.dram_tensor("dmask", [POS_PER_CORE, MPOS], f32, kind="ExternalInput")
    pc = nc.dram_tensor("pc", [1, 1], f32, kind="ExternalInput")
    pssm_o = nc.dram_tensor("pssm_part", [CNT_PER_CORE, NAA], f32, kind="ExternalOutput")
    cons_o = nc.dram_tensor("cons_part", [CNT_PER_CORE, 1], f32, kind="ExternalOutput")
    mi_o = nc.dram_tensor("mi_part", [POS_PER_CORE, MPOS], f32, kind="ExternalOutput")
    with tile.TileContext(nc) as tc:
        with ExitStack() as ctx:
            _emit_kernel(nc, tc, ctx,
                         (msa100, msa_mi, msa_cnt, s_all, dmask, pc,
                          pssm_o, cons_o, mi_o))
    nc.compile()
    _NC_CACHE = nc
    return nc


def _host_inputs(msa, pc):
    msa = np.ascontiguousarray(np.asarray(msa), dtype=np.int32)
    pc_np = np.asarray(pc, dtype=np.float32).reshape(1, 1)
    s_arr = np.zeros((P, 12), np.float32)
    for m in range(6):
        s_arr[NW * m: NW * m + NAA, m] = 1.0
        s_arr[NW * m + NAA, 6 + m] = 1.0
    msa100 = np.ascontiguousarray(msa[:, :MPOS])
    in_maps = []
    for c in range(NCORES):
        cols = [(POS_PER_CORE * c + t) if (POS_PER_CORE * c + t) < MPOS else 0
                for t in range(POS_PER_CORE)]
        dmask = np.full((POS_PER_CORE, MPOS), 1.0 / LN2, np.float32)
        for t in range(POS_PER_CORE):
            g = POS_PER_CORE * c + t
            if g < MPOS:
                dmask[t, g] = 0.0
        in_maps.append({
            "msa100": msa100,
            "msa_mi": np.ascontiguousarray(msa[:, cols]),
            "msa_cnt": np.ascontiguousarray(msa[:, CNT_PER_CORE * c: CNT_PER_CORE * (c + 1)]),
            "s_all": s_arr,
            "dmask": dmask,
            "pc": pc_np,
        })
    return in_maps


def _run(msa, pc, **spmd_kwargs):
    nc = _build_nc()
    in_maps = _host_inputs(msa, pc)
    res = run_bass_kernel_spmd(nc, in_maps, core_ids=list(range(NCORES)), **spmd_kwargs)
    pssm = np.concatenate([res.results[c]["pssm_part"] for c in range(NCORES)], axis=0)
    cons = np.concatenate([res.results[c]["cons_part"][:, 0] for c in range(NCORES)], axis=0)
    rows = np.concatenate([res.results[c]["mi_part"] for c in range(NCORES)], axis=0)
    mi = np.zeros((SEQ_LEN, SEQ_LEN), np.float32)
    mi[:MPOS, :MPOS] = rows[:MPOS]
    return (pssm.astype(np.float32), cons.astype(np.float32), mi), res


def kernel(msa, pc):
    out, _ = _run(msa, pc)
    return out
